# revision 8
# baseline (speedup 1.0000x reference)
"""AdaIN statistics kernel for TRN2, SPMD across 8 NeuronCores.

Input : f_vol [32, 512, 64, 64] f32
Output: [32, 1024] f32 = concat([mean over (h,w), unbiased std over (h,w)], axis=-1)

Sharding: data-parallel over batch — each of the 8 cores handles 4 batches
([4, 512, 64, 64] shard, 32 MiB). No collectives; the host concatenates the
8 per-core [4, 1024] outputs.

Per core: view the shard as 2048 rows (b*512+c) x 4096 spatial elems.
16 tiles of [128 rows, 4096]. Pipeline (raw Bass, manual semaphores —
Tile's scheduler emits 2 sync-waits on slot-reuse DMAs, which this
compiler's static-DMA encoding cannot hold):
  SP   : 16x 2 MiB input DMAs (ring of 8 SBUF slots)
  DVE  : 8x bn_stats + bn_aggr per tile -> (mean, biased var)
  ACT  : mean copy + sqrt(var * N/(N-1)), then the tiny output DMAs

DMA completion is not FIFO across in-flight transfers, so each input ring
slot / res slot gets its own DMA-completion semaphore (mirrors Tile's
DMASW lanes). Every cross-instruction data edge is covered by an explicit
semaphore observation so the CoreSim race detector can verify the design.
"""

from contextlib import ExitStack

import numpy as np

B, C, H, W = 32, 512, 64, 64
N_CORES = 8
B_LOCAL = B // N_CORES  # 4
N = H * W  # 4096
P = 128
ROWS = B_LOCAL * C  # 2048
NTILES = ROWS // P  # 16
TPB = C // P  # tiles (channel blocks) per batch = 4
G = N // 512  # bn_stats groups per row = 8
NBUF = 8  # input ring slots (8 x 16 KiB/partition)
NSMALL = 4  # stats/mv/res ring slots

_CACHE = {}


def _build():
    import concourse.bass as bass
    from concourse import mybir

    nc = bass.Bass()
    x_ext = nc.declare_dram_parameter(
        "f_vol", [B_LOCAL, C, H, W], mybir.dt.float32, isOutput=False
    )
    out_ext = nc.declare_dram_parameter(
        "out", [B_LOCAL, 2 * C], mybir.dt.float32, isOutput=True
    )

    x = x_ext.ap().rearrange("b c h w -> (b c) (h w)")  # [2048, 4096]
    o3 = out_ext.ap().rearrange("b (s c) -> b s c", s=2)  # [4, 2, 512]

    with ExitStack() as ctx:
        block = ctx.enter_context(nc.Block())
        dma_in = [ctx.enter_context(nc.semaphore(f"dma_in{k}")) for k in range(NBUF)]
        dma_out = [
            ctx.enter_context(nc.semaphore(f"dma_out{k}")) for k in range(NSMALL)
        ]
        dve_stats = ctx.enter_context(nc.semaphore("dve_stats"))  # +1 per bn_stats
        mv_ready = ctx.enter_context(nc.semaphore("mv_ready"))  # +1 per bn_aggr
        act_done = ctx.enter_context(nc.semaphore("act_done"))  # +2 per tile (ACT)
        xt = ctx.enter_context(nc.sbuf_tensor("xt", [P, NBUF, N], mybir.dt.float32))
        stats = ctx.enter_context(
            nc.sbuf_tensor("stats", [P, NSMALL, G, 6], mybir.dt.float32)
        )
        mv = ctx.enter_context(nc.sbuf_tensor("mv", [P, NSMALL, 2], mybir.dt.float32))
        res = ctx.enter_context(
            nc.sbuf_tensor("res", [P, NSMALL, 2], mybir.dt.float32)
        )

        @block.sync
        def _(sync):
            for t in range(NTILES):
                k = t % NBUF
                if t >= NBUF:
                    # slot k free once DVE's 8 bn_stats of tile t-NBUF retired,
                    # and the slot's previous DMA completion has been observed
                    sync.wait_ge(dve_stats, G * (t - NBUF + 1))
                    sync.wait_ge(dma_in[k], 16 * (t // NBUF))
                sync.dma_start(
                    out=xt[:, k, :], in_=x[t * P : (t + 1) * P, :]
                ).then_inc(dma_in[k], 16)
            # keep the NEFF alive until every output DMA has landed
            for s in range(NSMALL):
                sync.wait_ge(dma_out[s], 32 * (NTILES // NSMALL))

        @block.vector
        def _(vector):
            for t in range(NTILES):
                k = t % NBUF
                s = t % NSMALL
                vector.wait_ge(dma_in[k], 16 * (t // NBUF + 1))
                if t >= NSMALL:
                    # stats slot WAR: bn_aggr of tile t-NSMALL has read it
                    vector.wait_ge(mv_ready, t - NSMALL + 1)
                for g in range(G):
                    vector.bn_stats(
                        out=stats[:, s, g, :],
                        in_=xt[:, k, g * 512 : (g + 1) * 512],
                    ).then_inc(dve_stats, 1)
                if t >= NSMALL:
                    # mv slot WAR: ACT of tile t-NSMALL has read it
                    vector.wait_ge(act_done, 2 * (t - NSMALL) + 2)
                # stats RAW: all 8 bn_stats writes of THIS tile retired
                vector.wait_ge(dve_stats, G * (t + 1))
                vector.bn_aggr(out=mv[:, s, :], in_=stats[:, s, :, :]).then_inc(
                    mv_ready, 1
                )

        @block.scalar
        def _(scalar):
            for t in range(NTILES):
                s = t % NSMALL
                b, cb = divmod(t, TPB)
                scalar.wait_ge(mv_ready, t + 1)
                if t >= NSMALL:
                    # res slot reused: out-DMAs of tile t-NSMALL must be done
                    scalar.wait_ge(dma_out[s], 32 * (t // NSMALL))
                scalar.copy(out=res[:, s, 0:1], in_=mv[:, s, 0:1]).then_inc(
                    act_done, 1
                )
                scalar.activation(
                    out=res[:, s, 1:2],
                    in_=mv[:, s, 1:2],
                    func=mybir.ActivationFunctionType.Sqrt,
                    scale=float(N) / (N - 1),
                ).then_inc(act_done, 1)
                # res RAW: both ACT writes retired before the DMAs read them
                scalar.wait_ge(act_done, 2 * t + 2)
                # means -> out[b, cb*128 : (cb+1)*128], stds -> out[b, 512+...]
                scalar.dma_start(
                    out=o3[b, 0, cb * P : (cb + 1) * P], in_=res[:, s, 0:1]
                ).then_inc(dma_out[s], 16)
                scalar.dma_start(
                    out=o3[b, 1, cb * P : (cb + 1) * P], in_=res[:, s, 1:2]
                ).then_inc(dma_out[s], 16)

    return nc


def kernel(f_vol: np.ndarray) -> np.ndarray:
    from concourse.bass_utils import run_bass_kernel_spmd

    if "nc" not in _CACHE:
        _CACHE["nc"] = _build()
    nc = _CACHE["nc"]

    f_vol = np.ascontiguousarray(f_vol, dtype=np.float32)
    in_maps = [
        {"f_vol": f_vol[i * B_LOCAL : (i + 1) * B_LOCAL]} for i in range(N_CORES)
    ]
    res = run_bass_kernel_spmd(nc, in_maps, core_ids=list(range(N_CORES)))
    return np.concatenate([res.results[i]["out"] for i in range(N_CORES)], axis=0)
